# revision 1
# baseline (speedup 1.0000x reference)
"""Trainium2 Bass kernel for nn_FNNAttenModel (dense transformer: emb -> MHA -> tanh MLP -> decoder).

Sharding (8 NeuronCores, one chip):
  - Attention phase: data-parallel over batch (8 batches/core). Embedding gather via
    indirect DMA, Q/K computed transposed (weights stationary, batched moving operand),
    V/O computed with activation-stationary matmuls. No softmax max-subtraction needed
    (energies are tiny: inputs ~U(-0.1,0.1)).
  - Exchange: AllToAll re-shards the attention output from batch-sharded to
    d-sharded (each core gets x[:, :, 128i:128(i+1)] for ALL 64 batches, 2.1 MB bf16).
  - Hidden layer: contraction over (s, d in shard) with wh shard; partial h summed
    with a small AllReduce ([1024, 64] f32, 256 KB).
  - Decoder: vocab-sharded (4000 rows/core); host concatenates the 8 output shards.

All big matmuls in bf16 with fp32 PSUM accumulation; softmax/tanh/normalization in fp32.
"""

import sys
import os

for _p in ("/opt/trn_rl_repo", "/root/.axon_site/_ro/trn_rl_repo"):
    if os.path.isdir(_p) and _p not in sys.path:
        sys.path.insert(0, _p)

import numpy as np
import ml_dtypes
_KSKIP = os.environ.get('KSKIP', '')
_KXODC = os.environ.get('KXODC', '') == '1'

import concourse.bass as bass
import concourse.tile as tile
from concourse import bacc, mybir
from concourse.bass import IndirectOffsetOnAxis
from concourse.bass_utils import run_bass_kernel_spmd
from concourse.masks import make_identity

BF16 = mybir.dt.bfloat16
F32 = mybir.dt.float32
I32 = mybir.dt.int32

S = 128          # sequence length
B = 64           # total batch
D = 1024         # model dim
H = 16           # heads
HD = 64          # head dim
NT = 32000       # vocab
NH = 1024        # hidden
NCORES = 8
BPC = B // NCORES        # batches per core
DPC = D // NCORES        # d-shard width per core (exchange shard)
JT = NH // 128           # hidden-dim tiles
VPC = NT // NCORES       # vocab per core
NCH = 8                  # decoder N chunks per core
VCH = VPC // NCH         # 500, <= 512 (one PSUM bank in fp32)
INV_SCALE = 0.25         # 1/sqrt(nheads) = 1/4

RG = [list(range(NCORES))]

_CACHE = {}


def _bf16(x):
    return np.ascontiguousarray(x.astype(ml_dtypes.bfloat16))


def _f32(x):
    return np.ascontiguousarray(x.astype(np.float32))


def build_program(repeat: int = 1, no_collectives: bool = False, phases: str = "ABC", wh_dma="sync", wd_dma="sync", a2a_parts: int = 2):
    nc = bacc.Bacc(
        "TRN2",
        target_bir_lowering=False,
        debug=False,
        enable_asserts=False,
        num_devices=NCORES,
    )

    # ---------------- kernel I/O ----------------
    emb_t = nc.dram_tensor("emb", [NT, D], BF16, kind="ExternalInput")
    ids_t = nc.dram_tensor("ids", [S, BPC], I32, kind="ExternalInput")
    wqT_t = nc.dram_tensor("wqT", [8, 128, 1024], BF16, kind="ExternalInput")
    wkT_t = nc.dram_tensor("wkT", [8, 128, 1024], BF16, kind="ExternalInput")
    wvT_t = nc.dram_tensor("wvT", [8, 128, D], BF16, kind="ExternalInput")
    woT_t = nc.dram_tensor("woT", [8, 128, D], BF16, kind="ExternalInput")
    bq_t = nc.dram_tensor("bq", [128, 8], F32, kind="ExternalInput")
    bk_t = nc.dram_tensor("bk", [128, 8], F32, kind="ExternalInput")
    bv_t = nc.dram_tensor("bv", [1, D], BF16, kind="ExternalInput")
    bo_t = nc.dram_tensor("bo", [1, D], BF16, kind="ExternalInput")
    wh_t = nc.dram_tensor("wh", [DPC, S, NH], BF16, kind="ExternalInput")
    bh_t = nc.dram_tensor("bh", [128, JT], F32, kind="ExternalInput")
    wdT_t = nc.dram_tensor("wdT", [8, 128, VPC], BF16, kind="ExternalInput")
    bd_t = nc.dram_tensor("bd", [1, VPC], BF16, kind="ExternalInput")
    dec_t = nc.dram_tensor("dec", [B, VPC], F32, kind="ExternalOutput")

    emb_ap = emb_t.ap()
    wh_ap = wh_t.ap()
    wdT_ap = wdT_t.ap()
    dec_ap = dec_t.ap()

    with tile.TileContext(nc) as tc:
        # DRAM scratch (tile-tracked so the collectives order after their producers)
        with tc.tile_pool(name="dram", bufs=1, space="DRAM") as dram_pool:

            # persistent consts
            with tc.tile_pool(name="const", bufs=1) as constp:
                ident = constp.tile([128, 128], BF16, tag="ident")
                make_identity(nc, ident[:])
                ids_sb = constp.tile([S, BPC], I32, tag="ids")
                nc.sync.dma_start(ids_sb[:], ids_t.ap())
                bq_sb = constp.tile([128, 8], F32, tag="bq")
                nc.sync.dma_start(bq_sb[:], bq_t.ap())
                bk_sb = constp.tile([128, 8], F32, tag="bk")
                nc.sync.dma_start(bk_sb[:], bk_t.ap())
                bv_sb = constp.tile([1, D], BF16, tag="bv")
                nc.sync.dma_start(bv_sb[:], bv_t.ap())
                bo_sb = constp.tile([1, D], BF16, tag="bo")
                nc.sync.dma_start(bo_sb[:], bo_t.ap())
                bh_sb = constp.tile([128, JT], F32, tag="bh")
                nc.sync.dma_start(bh_sb[:], bh_t.ap())
                bd_sb = constp.tile([1, VPC], BF16, tag="bd")
                nc.sync.dma_start(bd_sb[:], bd_t.ap())
                ones1 = constp.tile([1, 128], BF16, tag="ones1")
                nc.vector.memset(ones1[:], 1.0)
                identf = constp.tile([128, 128], F32, tag="identf")
                make_identity(nc, identf[:])

                # weight-stream pools opened for the whole kernel so their SBUF
                # ranges don't alias phase-A tiles: lets wh/wd prefetch DMAs run
                # during the attention phase
                whp = tc.alloc_tile_pool(name="whp", bufs=5)
                wdp = tc.alloc_tile_pool(name="wdp", bufs=4)

                for rep in range(repeat):

                    BPP = BPC // a2a_parts
                    a2a_in = [dram_pool.tile([NCORES, S, BPP, DPC], BF16, tag=f"a2a_in{rep}_{h}", name=f"a2a_in{rep}_{h}") for h in range(a2a_parts)]
                    a2a_out = [dram_pool.tile([NCORES, S, BPP, DPC], BF16, tag=f"a2a_out{rep}_{h}", name=f"a2a_out{rep}_{h}") for h in range(a2a_parts)]
                    ar_in = dram_pool.tile([B, NH], F32, tag=f"ar_in{rep}", name=f"ar_in{rep}")
                    ar_out = dram_pool.tile([B, NH], F32, tag=f"ar_out{rep}", name=f"ar_out{rep}", addr_space="Shared")

                    if "A" in phases:
                        # ---------------- phase A: attention (batch-sharded) ----------------
                        with tc.tile_pool(name="phaseA", bufs=1) as pa, \
                             tc.tile_pool(name="gather", bufs=2) as gp, \
                             tc.tile_pool(name="wqk", bufs=2) as wqkp, \
                             tc.tile_pool(name="exp", bufs=3) as expp, \
                             tc.tile_pool(name="rec", bufs=3) as recp, \
                             tc.tile_pool(name="xo2", bufs=2) as xo2p, \
                             tc.tile_pool(name="psA", bufs=1, space="PSUM") as psA:

                            embT = [pa.tile([128, BPC * S], BF16, tag=f"embt{k}", name=f"embt{k}")
                                    for k in range(8)]
                            QT = [pa.tile([128, BPC * S], BF16, tag=f"qt{m}", name=f"qt{m}")
                                  for m in range(8)]
                            KT = [pa.tile([128, BPC * S], BF16, tag=f"kt{m}", name=f"kt{m}")
                                  for m in range(8)]
                            Vp = [pa.tile([128, H * (HD + 1)], BF16, tag=f"vp{b}", name=f"vp{b}")
                                  for b in range(BPC)]
                            xT = [[pa.tile([128, S], BF16, tag=f"xt{b}_{k}", name=f"xt{b}_{k}")
                                   for k in range(8)] for b in range(BPC)]
                            x_sb = [pa.tile([S, D], BF16, tag=f"xsb{b}", name=f"xsb{b}")
                                    for b in range(BPC)]
                            wv_sb = [pa.tile([128, D], BF16, tag=f"wv{k}", name=f"wv{k}")
                                     for k in range(8)]
                            wo_sb = [pa.tile([128, D], BF16, tag=f"wo{k}", name=f"wo{k}")
                                     for k in range(8)]
                            for k in range(8):
                                nc.sync.dma_start(wv_sb[k][:], wvT_t.ap()[k])
                                nc.sync.dma_start(wo_sb[k][:], woT_t.ap()[k])

                            # gather + transpose embeddings
                            for b in range(BPC):
                                g = gp.tile([S, D], BF16, tag="emb")
                                nc.gpsimd.indirect_dma_start(
                                    out=g[:],
                                    out_offset=None,
                                    in_=emb_ap,
                                    in_offset=IndirectOffsetOnAxis(ap=ids_sb[:, b : b + 1], axis=0),
                                )
                                for k in range(8):
                                    pt = psA.tile([128, 128], BF16, tag="attn", bufs=4)
                                    nc.tensor.transpose(pt[:], g[:, k * 128 : (k + 1) * 128], ident[:])
                                    nc.vector.tensor_copy(embT[k][:, b * S : (b + 1) * S], pt[:])

                            # Q / K projections: out = w @ emb.T for all batches at once
                            for m in range(8):
                                psq = psA.tile([128, BPC * S], F32, tag="proj", bufs=2)
                                psk = psA.tile([128, BPC * S], F32, tag="proj", bufs=2)
                                wtq = wqkp.tile([128, 8 * 128], BF16, tag="wq")
                                nc.sync.dma_start(wtq[:], wqT_t.ap()[m])
                                wtk = wqkp.tile([128, 8 * 128], BF16, tag="wk")
                                nc.sync.dma_start(wtk[:], wkT_t.ap()[m])
                                for k in range(8):
                                    ksl = slice(k * 128, (k + 1) * 128)
                                    for hf in range(2):
                                        sl = slice(hf * 512, (hf + 1) * 512)
                                        nc.tensor.matmul(
                                            psq[:, sl], wtq[:, ksl], embT[k][:, sl],
                                            start=(k == 0), stop=(k == 7),
                                        )
                                        nc.tensor.matmul(
                                            psk[:, sl], wtk[:, ksl], embT[k][:, sl],
                                            start=(k == 0), stop=(k == 7),
                                        )
                                nc.scalar.activation(
                                    QT[m][:], psq[:], mybir.ActivationFunctionType.Identity,
                                    bias=bq_sb[:, m : m + 1],
                                )
                                nc.vector.tensor_scalar_add(KT[m][:], psk[:], bk_sb[:, m : m + 1])

                            # V projection per batch (emb.T stationary, wv.T moving);
                            # bias accumulated via ones-column matmul
                            for b in range(BPC):
                                psv = psA.tile([S, D], F32, tag="proj", bufs=2)
                                for k in range(8):
                                    for hf in range(2):
                                        sl = slice(hf * 512, (hf + 1) * 512)
                                        nc.tensor.matmul(
                                            psv[:, sl],
                                            embT[k][:, b * S : (b + 1) * S],
                                            wv_sb[k][:, sl],
                                            start=(k == 0), stop=False,
                                        )
                                for hf in range(2):
                                    sl = slice(hf * 512, (hf + 1) * 512)
                                    nc.tensor.matmul(
                                        psv[:, sl], ones1[:, :S], bv_sb[:1, sl],
                                        start=False, stop=True,
                                    )
                                vp3 = Vp[b][:].rearrange("p (h c) -> p h c", h=H)
                                nc.vector.tensor_copy(
                                    vp3[:, :, 0:HD], psv[:].rearrange("p (h c) -> p h c", h=H)
                                )
                                nc.vector.memset(vp3[:, :, HD : HD + 1], 1.0)

                            # attention per (batch, head)
                            for b in range(BPC):
                                for h in range(H):
                                    m, hf = h // 2, h % 2
                                    prow = slice(hf * HD, (hf + 1) * HD)
                                    csl = slice(b * S, (b + 1) * S)
                                    pe = psA.tile([S, S], F32, tag="attn", bufs=4)
                                    # energy.T = K_h @ Q_h.T  (contraction over head dim, K=64)
                                    nc.tensor.matmul(
                                        pe[:], KT[m][prow, csl], QT[m][prow, csl],
                                        start=True, stop=True,
                                    )
                                    ex = expp.tile([S, S], BF16, tag="exp")
                                    nc.scalar.activation(
                                        ex[:], pe[:], mybir.ActivationFunctionType.Exp,
                                        scale=INV_SCALE,
                                    )
                                    # x_h' = exp(E.T).T @ [V_h | 1]: cols 0..63 = unnormalized
                                    # attn@V, col 64 = softmax denominators (per s_q partition)
                                    px = psA.tile([S, HD + 1], F32, tag="attn", bufs=4)
                                    nc.tensor.matmul(
                                        px[:], ex[:], Vp[b][:, h * (HD + 1) : (h + 1) * (HD + 1)],
                                        start=True, stop=True,
                                    )
                                    r = recp.tile([S, 1], F32, tag="rec")
                                    nc.vector.reciprocal(r[:], px[:, HD : HD + 1])
                                    nc.vector.tensor_scalar_mul(
                                        x_sb[b][:, h * HD : (h + 1) * HD], px[:, 0:HD], r[:]
                                    )
                                # transpose x for the output projection
                                for k in range(8):
                                    ptx = psA.tile([128, 128], BF16, tag="attn", bufs=4)
                                    nc.tensor.transpose(
                                        ptx[:], x_sb[b][:, k * 128 : (k + 1) * 128], ident[:]
                                    )
                                    nc.vector.tensor_copy(xT[b][k][:], ptx[:])

                            # output projection per batch (x.T stationary, wo.T moving)
                            for b in range(BPC):
                                pso = psA.tile([S, D], F32, tag="proj", bufs=2)
                                for k in range(8):
                                    for hf in range(2):
                                        sl = slice(hf * 512, (hf + 1) * 512)
                                        nc.tensor.matmul(
                                            pso[:, sl], xT[b][k][:], wo_sb[k][:, sl],
                                            start=(k == 0), stop=False,
                                        )
                                for hf in range(2):
                                    sl = slice(hf * 512, (hf + 1) * 512)
                                    nc.tensor.matmul(
                                        pso[:, sl], ones1[:, :S], bo_sb[:1, sl],
                                        start=False, stop=True,
                                    )
                                xo2 = xo2p.tile([S, D], BF16, tag="xo2")
                                nc.vector.tensor_copy(xo2[:], pso[:])
                                # scatter the 8 d-shards of this batch into the AllToAll input
                                nc.sync.dma_start(
                                    a2a_in[b // BPP][:, :, b % BPP].transpose([1, 0, 2]),
                                    xo2[:].rearrange("p (j d) -> p j d", j=NCORES),
                                )

                    if "B" in phases:
                        # ---------------- exchange: batch-shard -> d-shard ----------------
                        for h in range(a2a_parts):
                            if no_collectives:
                                nc.sync.dma_start(a2a_out[h][:], a2a_in[h][:])
                            else:
                                nc.gpsimd.collective_compute(
                                    "AllToAll",
                                    mybir.AluOpType.bypass,
                                    replica_groups=RG,
                                    ins=[a2a_in[h][:]],
                                    outs=[a2a_out[h][:]],
                                )

                        # ---------------- phase B: hidden layer (d-sharded) ----------------
                        with tc.tile_pool(name="phaseB", bufs=1) as pb, \
                             tc.tile_pool(name="hpart", bufs=2) as hpp, \
                             tc.tile_pool(name="psB", bufs=1, space="PSUM") as psB:

                            xod = pb.tile([S, B * DPC], BF16, tag="xod", name="xod")
                            xod4 = xod[:].rearrange("p (g b d) -> p g b d", g=NCORES, b=BPC)
                            for h in range(a2a_parts):
                                nc.sync.dma_start(
                                    xod4[:, :, h * BPP : (h + 1) * BPP, :],
                                    a2a_out[h][:].transpose([1, 0, 2, 3]),
                                )
                            xod_r = xod[:].rearrange("p (b d) -> p b d", d=DPC)
                            hds = pb.tile([B, NH], F32, tag="hds", name="hds")

                            # wide-N hidden matmuls: stationary = x-slice [s, b], moving = wh[d] rows
                            DCH = 4  # d's per weight DMA (1 MB tiles)
                            psh = [psB.tile([B, 512], F32, tag=f"psh{c}", bufs=1, name=f"psh{c}")
                                   for c in range(2)]
                            for dc in range(DPC // DCH):
                                wt = whp.tile([S, DCH * NH], BF16, tag="wh")
                                nc.sync.dma_start(
                                    wt[:].rearrange("p (d j) -> p d j", d=DCH),
                                    wh_ap[dc * DCH : (dc + 1) * DCH].transpose([1, 0, 2]),
                                )
                                for dd in range(DCH):
                                    d = dc * DCH + dd
                                    if _KXODC:
                                        xs = pb.tile([S, B], BF16, tag="xs", bufs=4, name="xs")
                                        nc.vector.tensor_copy(xs[:], xod_r[:, :, d])
                                        lhs_d = xs[:]
                                    else:
                                        lhs_d = xod_r[:, :, d]
                                    for c in range(2):
                                        nc.tensor.matmul(
                                            psh[c][:],
                                            lhs_d,
                                            wt[:, dd * NH + c * 512 : dd * NH + (c + 1) * 512],
                                            start=(d == 0), stop=(d == DPC - 1),
                                        )
                            for c in range(2):
                                nc.vector.tensor_copy(hds[:, c * 512 : (c + 1) * 512], psh[c][:])

                        nc.sync.dma_start(ar_in[:], hds[:])
                        if no_collectives:
                            nc.sync.dma_start(ar_out[:], ar_in[:])
                        else:
                            nc.gpsimd.collective_compute(
                                "AllReduce",
                                mybir.AluOpType.add,
                                replica_groups=RG,
                                ins=[ar_in[:]],
                                outs=[ar_out[:]],
                            )

                    if "C" in phases:
                        # ---------------- phase C: tanh + decoder (vocab-sharded) ----------
                        with tc.tile_pool(name="phaseC", bufs=1) as pc, \
                             tc.tile_pool(name="hload", bufs=2) as hlp, \
                             tc.tile_pool(name="dout", bufs=2) as dop, \
                             tc.tile_pool(name="psC", bufs=1, space="PSUM") as psC:

                            ht = [pc.tile([128, B], BF16, tag=f"ht{k}", name=f"ht{k}")
                                  for k in range(JT)]
                            hsall = hlp.tile([B, NH], F32, tag="hs")
                            nc.sync.dma_start(hsall[:], ar_out[:])
                            for k in range(JT):
                                ptr = psC.tile([128, B], F32, tag="htr", bufs=2)
                                nc.tensor.transpose(
                                    ptr[:], hsall[:, k * 128 : (k + 1) * 128], identf[:B, :B]
                                )
                                nc.scalar.activation(
                                    ht[k][:], ptr[:], mybir.ActivationFunctionType.Tanh,
                                    bias=bh_sb[:, k : k + 1],
                                )
                            W2 = 2 * VCH
                            for n in range(4):
                                nsl = slice(n * W2, (n + 1) * W2)
                                pd = psC.tile([B, 1024], F32, tag="pd", bufs=2)
                                for k in range(JT):
                                    wdt = wdp.tile([128, W2], BF16, tag="wd")
                                    getattr(nc, wd_dma).dma_start(wdt[:], wdT_ap[k, :, nsl])
                                    for hf in range(2):
                                        nc.tensor.matmul(
                                            pd[:, hf * 512 : hf * 512 + VCH],
                                            ht[k][:], wdt[:, hf * VCH : (hf + 1) * VCH],
                                            start=(k == 0), stop=False,
                                        )
                                for hf in range(2):
                                    nc.tensor.matmul(
                                        pd[:, hf * 512 : hf * 512 + VCH], ones1[:, :B],
                                        bd_sb[:1, n * W2 + hf * VCH : n * W2 + (hf + 1) * VCH],
                                        start=False, stop=True,
                                    )
                                ds = dop.tile([B, W2], F32, tag="ds")
                                for hf in range(2):
                                    nc.vector.tensor_copy(
                                        ds[:, hf * VCH : (hf + 1) * VCH], pd[:, hf * 512 : hf * 512 + VCH]
                                    )
                                nc.sync.dma_start(dec_ap[:, nsl], ds[:])
                wdp.release()
                whp.release()

    nc.compile()
    return nc


def prepare_inputs(inputs):
    """Host-side sharding: returns per-core input maps."""
    ids_full = np.asarray(inputs["input"]).astype(np.int32)          # [S, B]
    emb_W = np.asarray(inputs["emb_W"], dtype=np.float32)
    wq = np.asarray(inputs["wq"], dtype=np.float32)
    wk = np.asarray(inputs["wk"], dtype=np.float32)
    wv = np.asarray(inputs["wv"], dtype=np.float32)
    wo = np.asarray(inputs["wo"], dtype=np.float32)
    wh = np.asarray(inputs["wh"], dtype=np.float32)                  # [NH, D*S]
    wd = np.asarray(inputs["wd"], dtype=np.float32)                  # [NT, NH]
    bq = np.asarray(inputs["bq"], dtype=np.float32)
    bk = np.asarray(inputs["bk"], dtype=np.float32)
    bv = np.asarray(inputs["bv"], dtype=np.float32)
    bo = np.asarray(inputs["bo"], dtype=np.float32)
    bh = np.asarray(inputs["bh"], dtype=np.float32)
    bd = np.asarray(inputs["bd"], dtype=np.float32)

    emb_bf = _bf16(emb_W)

    def qk_tiles_fixed(w):
        # w: [D_out, D_in] -> [mt, p, (kt q)]: [mt,p,kt,q] = w[mt*128+q, kt*128+p]
        a = w.reshape(8, 128, 8, 128)           # [mt, q, kt, p]
        return _bf16(a.transpose(0, 3, 2, 1).reshape(8, 128, 1024))

    def vo_tiles(w):
        # w.T reshaped [k, 128, D]: tile k holds rows k*128..k*128+127 of w.T
        return _bf16(w.T.reshape(8, 128, D))

    wqT = qk_tiles_fixed(wq)
    wkT = qk_tiles_fixed(wk)
    wvT = vo_tiles(wv)
    woT = vo_tiles(wo)

    bq_r = _f32(bq.reshape(8, 128).T)
    bk_r = _f32(bk.reshape(8, 128).T)
    bv_r = _bf16(bv.reshape(1, D))
    bo_r = _bf16(bo.reshape(1, D))
    bh_r = _f32(bh.reshape(JT, 128).T)

    # wh: [NH, S*D] with k = s*D + d  ->  per-core [jt, d_local, s, j]
    wh_r = wh.reshape(NH, S, D)                  # [j_full, s, d]

    in_maps = []
    for i in range(NCORES):
        ids_i = np.ascontiguousarray(ids_full[:, i * BPC : (i + 1) * BPC])
        whs = wh_r[:, :, i * DPC : (i + 1) * DPC]          # [NH, S, DPC]
        wh_i = _bf16(whs.transpose(2, 1, 0))               # [d, s, j]
        wd_i = wd[i * VPC : (i + 1) * VPC]                 # [VPC, NH]
        wdT_i = _bf16(wd_i.T.reshape(8, 128, VPC))
        bd_i = _bf16(bd[i * VPC : (i + 1) * VPC].reshape(1, VPC))
        in_maps.append(
            {
                "emb": emb_bf,
                "ids": ids_i,
                "wqT": wqT,
                "wkT": wkT,
                "wvT": wvT,
                "woT": woT,
                "bq": bq_r,
                "bk": bk_r,
                "bv": bv_r,
                "bo": bo_r,
                "wh": wh_i,
                "bh": bh_r,
                "wdT": wdT_i,
                "bd": bd_i,
            }
        )
    return in_maps


def _make_runner(nc, in_maps):
    """Build a jitted single-custom-call runner (mirrors bass2jax.run_bass_via_pjrt)
    with device-resident inputs. Returns (run_fn, fetch_outputs_fn)."""
    import jax
    from jax.experimental.shard_map import shard_map
    from jax.sharding import Mesh, PartitionSpec, NamedSharding
    from concourse import bass2jax
    from concourse.bass2jax import _bass_exec_p, partition_id_tensor

    bass2jax.install_neuronx_cc_hook()

    n_cores = len(in_maps)
    partition_name = nc.partition_id_tensor.name if nc.partition_id_tensor else None
    in_names, out_names, out_avals, zero_outs = [], [], [], []
    for alloc in nc.m.functions[0].allocations:
        if not isinstance(alloc, mybir.MemoryLocationSet):
            continue
        name = alloc.memorylocations[0].name
        if alloc.kind == "ExternalInput":
            if name != partition_name:
                in_names.append(name)
        elif alloc.kind == "ExternalOutput":
            out_names.append(name)
            shape = tuple(alloc.tensor_shape)
            dtype = mybir.dt.np(alloc.dtype)
            out_avals.append(jax.core.ShapedArray(shape, dtype))
            zero_outs.append(np.zeros(shape, dtype))
    n_params = len(in_names)
    all_in_names = list(in_names) + list(out_names)
    if partition_name is not None:
        all_in_names.append(partition_name)

    def _body(*args):
        operands = list(args)
        if partition_name is not None:
            operands.append(partition_id_tensor())
        return tuple(
            _bass_exec_p.bind(
                *operands,
                out_avals=tuple(out_avals),
                in_names=tuple(all_in_names),
                out_names=tuple(out_names),
                lowering_input_output_aliases=(),
                sim_require_finite=True,
                sim_require_nnan=True,
                nc=nc,
            )
        )

    devices = jax.devices()[:n_cores]
    mesh = Mesh(np.asarray(devices), ("core",))
    f = jax.jit(
        shard_map(
            _body, mesh=mesh,
            in_specs=(PartitionSpec("core"),) * (n_params + len(out_names)),
            out_specs=(PartitionSpec("core"),) * len(out_names),
            check_rep=False,
        ),
        keep_unused=True,
    )
    concat_in = [
        np.concatenate([np.asarray(in_maps[c][nm]) for c in range(n_cores)], axis=0)
        for nm in in_names
    ]
    concat_zeros = [
        np.zeros((n_cores * z.shape[0], *z.shape[1:]), z.dtype) for z in zero_outs
    ]
    sh = NamedSharding(mesh, PartitionSpec("core"))
    dev_in = [jax.device_put(a, sh) for a in concat_in]
    dev_zero = [jax.device_put(a, sh) for a in concat_zeros]

    def run():
        return jax.block_until_ready(f(*dev_in, *dev_zero))

    def fetch(out_arrs):
        return [
            {
                nm: np.asarray(out_arrs[i]).reshape(n_cores, *out_avals[i].shape)[c]
                for i, nm in enumerate(out_names)
            }
            for c in range(n_cores)
        ]

    return run, fetch


def run_timed(in_maps, iters=11, reps=5):
    """Time the kernel by comparing a repeat=1 program against a repeat=iters
    program (single custom call each; host dispatch cancels in the difference).
    Returns (outputs_per_core, secs_per_exec)."""
    import time

    nc1 = _CACHE.get("nc") or build_program(repeat=1)
    _CACHE["nc"] = nc1
    if "ncN" not in _CACHE or _CACHE.get("ncN_iters") != iters:
        _CACHE["ncN"] = build_program(repeat=iters)
        _CACHE["ncN_iters"] = iters
    run1, fetch1 = _make_runner(nc1, in_maps)
    runN, _ = _make_runner(_CACHE["ncN"], in_maps)

    out1 = run1()  # warm/compile
    runN()         # warm/compile

    best = None
    for _ in range(reps):
        t0 = time.perf_counter()
        run1()
        t1 = time.perf_counter()
        runN()
        t2 = time.perf_counter()
        per = ((t2 - t1) - (t1 - t0)) / (iters - 1)
        best = per if best is None else min(best, per)

    return fetch1(out1), best


def kernel(**inputs) -> np.ndarray:
    if "nc" not in _CACHE:
        _CACHE["nc"] = build_program()
    nc = _CACHE["nc"]
    in_maps = prepare_inputs(inputs)
    res = run_bass_kernel_spmd(nc, in_maps, core_ids=list(range(NCORES)))
    out = np.concatenate([res.results[i]["dec"] for i in range(NCORES)], axis=1)
    return np.ascontiguousarray(out.astype(np.float32))


if __name__ == "__main__":
    # smoke: build only
    build_program()
    print("build ok")



# revision 15
# speedup vs baseline: 1.8839x; 1.8839x over previous
"""Trainium2 Bass kernel for nn_FNNAttenModel (dense transformer: emb -> MHA -> tanh MLP -> decoder).

Sharding (8 NeuronCores, one chip):
  - Attention phase: data-parallel over batch (8 batches/core). Embedding gather via
    indirect DMA, Q/K computed transposed (weights stationary, batched moving operand),
    V/O computed with activation-stationary matmuls. No softmax max-subtraction needed
    (energies are tiny: inputs ~U(-0.1,0.1)).
  - Exchange: AllToAll re-shards the attention output from batch-sharded to
    d-sharded (each core gets x[:, :, 128i:128(i+1)] for ALL 64 batches, 2.1 MB bf16).
  - Hidden layer: contraction over (s, d in shard) with wh shard; partial h summed
    with a small AllReduce ([1024, 64] f32, 256 KB).
  - Decoder: vocab-sharded (4000 rows/core); host concatenates the 8 output shards.

All big matmuls in bf16 with fp32 PSUM accumulation; softmax/tanh/normalization in fp32.
"""

import sys
import os

for _p in ("/opt/trn_rl_repo", "/root/.axon_site/_ro/trn_rl_repo"):
    if os.path.isdir(_p) and _p not in sys.path:
        sys.path.insert(0, _p)

import numpy as np
import ml_dtypes
_KSKIP = os.environ.get('KSKIP', '')
_KXODC = os.environ.get('KXODC', '') == '1'

import concourse.bass as bass
import concourse.tile as tile
from concourse import bacc, mybir
from concourse.bass import IndirectOffsetOnAxis
from concourse.bass_utils import run_bass_kernel_spmd
from concourse.masks import make_identity

BF16 = mybir.dt.bfloat16
F32 = mybir.dt.float32
I32 = mybir.dt.int32
FP8 = mybir.dt.float8e4
DR = mybir.MatmulPerfMode.DoubleRow

S = 128          # sequence length
B = 64           # total batch
D = 1024         # model dim
H = 16           # heads
HD = 64          # head dim
NT = 32000       # vocab
NH = 1024        # hidden
NCORES = 8
BPC = B // NCORES        # batches per core
DPC = D // NCORES        # d-shard width per core (exchange shard)
JT = NH // 128           # hidden-dim tiles
VPC = NT // NCORES       # vocab per core
NCH = 8                  # decoder N chunks per core
VCH = VPC // NCH         # 500, <= 512 (one PSUM bank in fp32)
INV_SCALE = 0.25         # 1/sqrt(nheads) = 1/4

RG = [list(range(NCORES))]

_CACHE = {}


def _bf16(x):
    return np.ascontiguousarray(x.astype(ml_dtypes.bfloat16))


def _f32(x):
    return np.ascontiguousarray(x.astype(np.float32))


def build_program(repeat: int = 1, no_collectives: bool = False, phases: str = "ABC", wh_dma="sync", wd_dma="sync", a2a_parts: int = 2):
    nc = bacc.Bacc(
        "TRN2",
        target_bir_lowering=False,
        debug=False,
        enable_asserts=False,
        num_devices=NCORES,
    )

    # ---------------- kernel I/O ----------------
    emb_t = nc.dram_tensor("emb", [NT, D], BF16, kind="ExternalInput")
    ids_t = nc.dram_tensor("ids", [S, BPC], I32, kind="ExternalInput")
    wqT_t = nc.dram_tensor("wqT", [8, 128, 1024], FP8, kind="ExternalInput")
    wkT_t = nc.dram_tensor("wkT", [8, 128, 1024], FP8, kind="ExternalInput")
    wvT_t = nc.dram_tensor("wvT", [8, 128, D], BF16, kind="ExternalInput")
    woT_t = nc.dram_tensor("woT", [8, 128, D], BF16, kind="ExternalInput")
    bq_t = nc.dram_tensor("bq", [128, 8], F32, kind="ExternalInput")
    bk_t = nc.dram_tensor("bk", [128, 8], F32, kind="ExternalInput")
    wh_t = nc.dram_tensor("wh", [DPC, S, NH], BF16, kind="ExternalInput")
    bh_t = nc.dram_tensor("bh", [128, JT], F32, kind="ExternalInput")
    wdT_t = nc.dram_tensor("wdT", [8, 128, VPC], BF16, kind="ExternalInput")
    bd_t = nc.dram_tensor("bd", [1, VPC], BF16, kind="ExternalInput")
    dec_t = nc.dram_tensor("dec", [B, VPC], F32, kind="ExternalOutput")

    emb_ap = emb_t.ap()
    wh_ap = wh_t.ap()
    wdT_ap = wdT_t.ap()
    dec_ap = dec_t.ap()

    with tile.TileContext(nc) as tc:
        # DRAM scratch (tile-tracked so the collectives order after their producers)
        with tc.tile_pool(name="dram", bufs=1, space="DRAM") as dram_pool:

            # persistent consts
            with tc.tile_pool(name="const", bufs=1) as constp:
                ident = constp.tile([128, 128], BF16, tag="ident")
                make_identity(nc, ident[:])
                ids_sb = constp.tile([S, BPC], I32, tag="ids")
                nc.sync.dma_start(ids_sb[:], ids_t.ap())
                bq_sb = constp.tile([128, 8], F32, tag="bq")
                nc.sync.dma_start(bq_sb[:], bq_t.ap())
                bk_sb = constp.tile([128, 8], F32, tag="bk")
                nc.sync.dma_start(bk_sb[:], bk_t.ap())
                bh_sb = constp.tile([128, JT], F32, tag="bh")
                nc.sync.dma_start(bh_sb[:], bh_t.ap())
                bd_sb = constp.tile([1, VPC], BF16, tag="bd")
                nc.sync.dma_start(bd_sb[:], bd_t.ap())
                ones1 = constp.tile([1, 128], BF16, tag="ones1")
                nc.vector.memset(ones1[:], 1.0)
                identf = constp.tile([128, 128], F32, tag="identf")
                make_identity(nc, identf[:])

                # weight-stream pools opened for the whole kernel so their SBUF
                # ranges don't alias phase-A tiles: lets wh/wd prefetch DMAs run
                # during the attention phase
                whp = tc.alloc_tile_pool(name="whp", bufs=4)
                wdp = tc.alloc_tile_pool(name="wdp", bufs=4)

                for rep in range(repeat):

                    BPP = BPC // a2a_parts
                    a2a_in = [dram_pool.tile([NCORES, S, BPP, DPC], BF16, tag=f"a2a_in{rep}_{h}", name=f"a2a_in{rep}_{h}") for h in range(a2a_parts)]
                    a2a_out = [dram_pool.tile([NCORES, S, BPP, DPC], BF16, tag=f"a2a_out{rep}_{h}", name=f"a2a_out{rep}_{h}") for h in range(a2a_parts)]
                    ar_in = dram_pool.tile([B, NH], F32, tag=f"ar_in{rep}", name=f"ar_in{rep}")
                    ar_out = dram_pool.tile([B, NH], F32, tag=f"ar_out{rep}", name=f"ar_out{rep}", addr_space="Shared")

                    if "A" in phases:
                        # ---------------- phase A: attention (batch-sharded) ----------------
                        with tc.tile_pool(name="phaseA", bufs=1) as pa, \
                             tc.tile_pool(name="gather", bufs=2) as gp, \
                             tc.tile_pool(name="wqk", bufs=2) as wqkp, \
                             tc.tile_pool(name="exp", bufs=3) as expp, \
                             tc.tile_pool(name="rec", bufs=3) as recp, \
                             tc.tile_pool(name="xo2", bufs=2) as xo2p, \
                             tc.tile_pool(name="psA", bufs=1, space="PSUM") as psA:

                            embT = [pa.tile([128, BPC * S], BF16, tag=f"embt{k}", name=f"embt{k}")
                                    for k in range(8)]
                            embT8 = pa.tile([128, 8 * BPC * S], FP8, tag="embt8", name="embt8")
                            embT8_r = embT8[:].rearrange("p (k f) -> p k f", k=8)
                            QT = [pa.tile([128, BPC * S], BF16, tag=f"qt{m}", name=f"qt{m}")
                                  for m in range(8)]
                            KT = [pa.tile([128, BPC * S], BF16, tag=f"kt{m}", name=f"kt{m}")
                                  for m in range(8)]
                            Vp = [pa.tile([128, H * (HD + 1)], BF16, tag=f"vp{b}", name=f"vp{b}")
                                  for b in range(BPC)]
                            xT = [[pa.tile([128, S], BF16, tag=f"xt{b}_{k}", name=f"xt{b}_{k}")
                                   for k in range(8)] for b in range(BPC)]
                            x_sb = [pa.tile([S, D], BF16, tag=f"xsb{b}", name=f"xsb{b}")
                                    for b in range(BPC)]
                            wv_sb = [pa.tile([128, D], BF16, tag=f"wv{k}", name=f"wv{k}")
                                     for k in range(8)]
                            wo_sb = [pa.tile([128, D], BF16, tag=f"wo{k}", name=f"wo{k}")
                                     for k in range(8)]
                            for k in range(8):
                                nc.sync.dma_start(wv_sb[k][:], wvT_t.ap()[k])
                                nc.sync.dma_start(wo_sb[k][:], woT_t.ap()[k])

                            # gather + transpose embeddings
                            for b in range(BPC):
                                g = gp.tile([S, D], BF16, tag="emb")
                                nc.gpsimd.indirect_dma_start(
                                    out=g[:],
                                    out_offset=None,
                                    in_=emb_ap,
                                    in_offset=IndirectOffsetOnAxis(ap=ids_sb[:, b : b + 1], axis=0),
                                )
                                for k in range(8):
                                    pt = psA.tile([128, 128], BF16, tag="attn", bufs=4)
                                    nc.tensor.transpose(pt[:], g[:, k * 128 : (k + 1) * 128], ident[:])
                                    nc.vector.tensor_copy(embT[k][:, b * S : (b + 1) * S], pt[:])
                            # fp8 copies of emb.T for the QK projections (ACT engine)
                            for k in range(8):
                                nc.scalar.copy(embT8_r[:, k, :], embT[k][:])

                            # V projection per batch (emb.T stationary, wv.T moving);
                            # biases folded into bh on host
                            for b in range(BPC):
                                psv = psA.tile([S, D], F32, tag="proj", bufs=2)
                                for k in range(8):
                                    for hf in range(2):
                                        sl = slice(hf * 512, (hf + 1) * 512)
                                        nc.tensor.matmul(
                                            psv[:, sl],
                                            embT[k][:, b * S : (b + 1) * S],
                                            wv_sb[k][:, sl],
                                            start=(k == 0), stop=(k == 7),
                                        )
                                vp3 = Vp[b][:].rearrange("p (h c) -> p h c", h=H)
                                nc.vector.tensor_copy(
                                    vp3[:, :, 0:HD], psv[:].rearrange("p (h c) -> p h c", h=H)
                                )
                                nc.vector.memset(vp3[:, :, HD : HD + 1], 1.0)

                            # Q / K projections in fp8 DoubleRow (2 k-tiles per
                            # step, 0.5 cycles/row): out = w @ emb.T for all batches
                            for m in range(8):
                                psq = psA.tile([128, BPC * S], F32, tag="proj", bufs=2)
                                psk = psA.tile([128, BPC * S], F32, tag="proj", bufs=2)
                                wtq = wqkp.tile([128, 1024], FP8, tag="wq")
                                nc.sync.dma_start(wtq[:], wqT_t.ap()[m])
                                wtk = wqkp.tile([128, 1024], FP8, tag="wk")
                                nc.sync.dma_start(wtk[:], wkT_t.ap()[m])
                                wtq_r = wtq[:].rearrange("p (kk i q) -> p kk i q", kk=4, i=2)
                                wtk_r = wtk[:].rearrange("p (kk i q) -> p kk i q", kk=4, i=2)
                                for kk in range(4):
                                    for hf in range(2):
                                        sl = slice(hf * 512, (hf + 1) * 512)
                                        nc.tensor.matmul(
                                            psq[:, sl], wtq_r[:, kk],
                                            embT8_r[:, 2 * kk : 2 * kk + 2, sl],
                                            start=(kk == 0), stop=(kk == 3),
                                            perf_mode=DR,
                                        )
                                        nc.tensor.matmul(
                                            psk[:, sl], wtk_r[:, kk],
                                            embT8_r[:, 2 * kk : 2 * kk + 2, sl],
                                            start=(kk == 0), stop=(kk == 3),
                                            perf_mode=DR,
                                        )
                                nc.scalar.activation(
                                    QT[m][:], psq[:], mybir.ActivationFunctionType.Identity,
                                    bias=bq_sb[:, m : m + 1],
                                )
                                nc.vector.tensor_scalar_add(KT[m][:], psk[:], bk_sb[:, m : m + 1])

                            # attention per (batch, head)
                            for b in range(BPC):
                                for h in range(H):
                                    m, hf = h // 2, h % 2
                                    prow = slice(hf * HD, (hf + 1) * HD)
                                    csl = slice(b * S, (b + 1) * S)
                                    pe = psA.tile([S, S], F32, tag="attn", bufs=4)
                                    # energy.T = K_h @ Q_h.T  (contraction over head dim, K=64)
                                    nc.tensor.matmul(
                                        pe[:], KT[m][prow, csl], QT[m][prow, csl],
                                        start=True, stop=True,
                                    )
                                    ex = expp.tile([S, S], BF16, tag="exp")
                                    nc.scalar.activation(
                                        ex[:], pe[:], mybir.ActivationFunctionType.Exp,
                                        scale=INV_SCALE,
                                    )
                                    # x_h' = exp(E.T).T @ [V_h | 1]: cols 0..63 = unnormalized
                                    # attn@V, col 64 = softmax denominators (per s_q partition)
                                    px = psA.tile([S, HD + 1], F32, tag="attn", bufs=4)
                                    nc.tensor.matmul(
                                        px[:], ex[:], Vp[b][:, h * (HD + 1) : (h + 1) * (HD + 1)],
                                        start=True, stop=True,
                                    )
                                    r = recp.tile([S, 1], F32, tag="rec")
                                    nc.vector.reciprocal(r[:], px[:, HD : HD + 1])
                                    nc.vector.tensor_scalar_mul(
                                        x_sb[b][:, h * HD : (h + 1) * HD], px[:, 0:HD], r[:]
                                    )
                                # transpose x for the output projection
                                for k in range(8):
                                    ptx = psA.tile([128, 128], BF16, tag="attn", bufs=4)
                                    nc.tensor.transpose(
                                        ptx[:], x_sb[b][:, k * 128 : (k + 1) * 128], ident[:]
                                    )
                                    nc.vector.tensor_copy(xT[b][k][:], ptx[:])

                            # output projection per batch (x.T stationary, wo.T moving)
                            for b in range(BPC):
                                pso = psA.tile([S, D], F32, tag="proj", bufs=2)
                                for k in range(8):
                                    for hf in range(2):
                                        sl = slice(hf * 512, (hf + 1) * 512)
                                        nc.tensor.matmul(
                                            pso[:, sl], xT[b][k][:], wo_sb[k][:, sl],
                                            start=(k == 0), stop=(k == 7),
                                        )
                                xo2 = xo2p.tile([S, D], BF16, tag="xo2")
                                nc.vector.tensor_copy(xo2[:], pso[:])
                                # scatter the 8 d-shards of this batch into the AllToAll input
                                nc.sync.dma_start(
                                    a2a_in[b // BPP][:, :, b % BPP].transpose([1, 0, 2]),
                                    xo2[:].rearrange("p (j d) -> p j d", j=NCORES),
                                )

                    if "B" in phases:
                        # ---------------- exchange: batch-shard -> d-shard ----------------
                        for h in range(a2a_parts):
                            if no_collectives:
                                nc.sync.dma_start(a2a_out[h][:], a2a_in[h][:])
                            else:
                                nc.gpsimd.collective_compute(
                                    "AllToAll",
                                    mybir.AluOpType.bypass,
                                    replica_groups=RG,
                                    ins=[a2a_in[h][:]],
                                    outs=[a2a_out[h][:]],
                                )

                        # ---------------- phase B: hidden layer (d-sharded) ----------------
                        with tc.tile_pool(name="phaseB", bufs=1) as pb, \
                             tc.tile_pool(name="hpart", bufs=2) as hpp, \
                             tc.tile_pool(name="psB", bufs=1, space="PSUM") as psB:

                            xod = pb.tile([S, B * DPC], BF16, tag="xod", name="xod")
                            xod4 = xod[:].rearrange("p (g b d) -> p g b d", g=NCORES, b=BPC)
                            for h in range(a2a_parts):
                                nc.sync.dma_start(
                                    xod4[:, :, h * BPP : (h + 1) * BPP, :],
                                    a2a_out[h][:].transpose([1, 0, 2, 3]),
                                )
                            xod_r = xod[:].rearrange("p (b d) -> p b d", d=DPC)
                            hds = pb.tile([B, NH], F32, tag="hds", name="hds")

                            # wide-N hidden matmuls: stationary = x-slice [s, b], moving = wh[d] rows
                            DCH = 4  # d's per weight DMA (1 MB tiles)
                            psh = [psB.tile([B, 512], F32, tag=f"psh{c}", bufs=1, name=f"psh{c}")
                                   for c in range(2)]
                            for dc in range(DPC // DCH):
                                wt = whp.tile([S, DCH * NH], BF16, tag="wh")
                                nc.sync.dma_start(
                                    wt[:].rearrange("p (d j) -> p d j", d=DCH),
                                    wh_ap[dc * DCH : (dc + 1) * DCH].transpose([1, 0, 2]),
                                )
                                for dd in range(DCH):
                                    d = dc * DCH + dd
                                    if _KXODC:
                                        xs = pb.tile([S, B], BF16, tag="xs", bufs=4, name="xs")
                                        nc.vector.tensor_copy(xs[:], xod_r[:, :, d])
                                        lhs_d = xs[:]
                                    else:
                                        lhs_d = xod_r[:, :, d]
                                    for c in range(2):
                                        nc.tensor.matmul(
                                            psh[c][:],
                                            lhs_d,
                                            wt[:, dd * NH + c * 512 : dd * NH + (c + 1) * 512],
                                            start=(d == 0), stop=(d == DPC - 1),
                                        )
                            for c in range(2):
                                nc.vector.tensor_copy(hds[:, c * 512 : (c + 1) * 512], psh[c][:])

                        nc.sync.dma_start(ar_in[:], hds[:])
                        if no_collectives:
                            nc.sync.dma_start(ar_out[:], ar_in[:])
                        else:
                            nc.gpsimd.collective_compute(
                                "AllReduce",
                                mybir.AluOpType.add,
                                replica_groups=RG,
                                ins=[ar_in[:]],
                                outs=[ar_out[:]],
                            )

                    if "C" in phases:
                        # ---------------- phase C: tanh + decoder (vocab-sharded) ----------
                        with tc.tile_pool(name="phaseC", bufs=1) as pc, \
                             tc.tile_pool(name="hload", bufs=2) as hlp, \
                             tc.tile_pool(name="dout", bufs=2) as dop, \
                             tc.tile_pool(name="psC", bufs=1, space="PSUM") as psC:

                            ht = [pc.tile([128, B], BF16, tag=f"ht{k}", name=f"ht{k}")
                                  for k in range(JT)]
                            hsall = hlp.tile([B, NH], F32, tag="hs")
                            nc.sync.dma_start(hsall[:], ar_out[:])
                            for k in range(JT):
                                ptr = psC.tile([128, B], F32, tag="htr", bufs=2)
                                nc.tensor.transpose(
                                    ptr[:], hsall[:, k * 128 : (k + 1) * 128], identf[:B, :B]
                                )
                                nc.scalar.activation(
                                    ht[k][:], ptr[:], mybir.ActivationFunctionType.Tanh,
                                    bias=bh_sb[:, k : k + 1],
                                )
                            W2 = 2 * VCH
                            for n in range(4):
                                nsl = slice(n * W2, (n + 1) * W2)
                                pd = psC.tile([B, 1024], F32, tag="pd", bufs=2)
                                for k in range(JT):
                                    wdt = wdp.tile([128, W2], BF16, tag="wd")
                                    getattr(nc, wd_dma).dma_start(wdt[:], wdT_ap[k, :, nsl])
                                    for hf in range(2):
                                        nc.tensor.matmul(
                                            pd[:, hf * 512 : hf * 512 + VCH],
                                            ht[k][:], wdt[:, hf * VCH : (hf + 1) * VCH],
                                            start=(k == 0), stop=False,
                                        )
                                for hf in range(2):
                                    nc.tensor.matmul(
                                        pd[:, hf * 512 : hf * 512 + VCH], ones1[:, :B],
                                        bd_sb[:1, n * W2 + hf * VCH : n * W2 + (hf + 1) * VCH],
                                        start=False, stop=True,
                                    )
                                ds = dop.tile([B, W2], F32, tag="ds")
                                for hf in range(2):
                                    nc.vector.tensor_copy(
                                        ds[:, hf * VCH : (hf + 1) * VCH], pd[:, hf * 512 : hf * 512 + VCH]
                                    )
                                nc.sync.dma_start(dec_ap[:, nsl], ds[:])
                wdp.release()
                whp.release()

    nc.compile()
    return nc


def prepare_inputs(inputs):
    """Host-side sharding: returns per-core input maps."""
    ids_full = np.asarray(inputs["input"]).astype(np.int32)          # [S, B]
    emb_W = np.asarray(inputs["emb_W"], dtype=np.float32)
    wq = np.asarray(inputs["wq"], dtype=np.float32)
    wk = np.asarray(inputs["wk"], dtype=np.float32)
    wv = np.asarray(inputs["wv"], dtype=np.float32)
    wo = np.asarray(inputs["wo"], dtype=np.float32)
    wh = np.asarray(inputs["wh"], dtype=np.float32)                  # [NH, D*S]
    wd = np.asarray(inputs["wd"], dtype=np.float32)                  # [NT, NH]
    bq = np.asarray(inputs["bq"], dtype=np.float32)
    bk = np.asarray(inputs["bk"], dtype=np.float32)
    bv = np.asarray(inputs["bv"], dtype=np.float32)
    bo = np.asarray(inputs["bo"], dtype=np.float32)
    bh = np.asarray(inputs["bh"], dtype=np.float32)
    bd = np.asarray(inputs["bd"], dtype=np.float32)

    emb_bf = _bf16(emb_W)

    def _fp8(x):
        return np.ascontiguousarray(x.astype(ml_dtypes.float8_e4m3))

    def qk_tiles_dr(w):
        # fp8 DoubleRow layout: [mt, p, kk, i, q] = w[mt*128+q, (2kk+i)*128+p]
        a = w.reshape(8, 128, 4, 2, 128)        # [mt, q, kk, i, p]
        return _fp8(a.transpose(0, 4, 2, 3, 1).reshape(8, 128, 1024))

    def vo_tiles(w):
        # w.T reshaped [k, 128, D]: tile k holds rows k*128..k*128+127 of w.T
        return _bf16(w.T.reshape(8, 128, D))

    wqT = qk_tiles_dr(wq)
    wkT = qk_tiles_dr(wk)
    wvT = vo_tiles(wv)
    woT = vo_tiles(wo)

    bq_r = _f32(bq.reshape(8, 128).T)
    bk_r = _f32(bk.reshape(8, 128).T)
    # fold the V and O biases into the hidden bias:
    #   x_final = attnV @ wo.T + (wo @ bv + bo)  (attention rows sum to 1)
    #   h_pre  += sum_{s,d} c[d] * wh[j, s*D+d]
    c_vec = wo @ bv + bo                                 # [D]
    bh_eff = bh + wh.reshape(NH, S, D).sum(axis=1) @ c_vec
    bh_r = _f32(bh_eff.reshape(JT, 128).T)

    # wh: [NH, S*D] with k = s*D + d  ->  per-core [jt, d_local, s, j]
    wh_r = wh.reshape(NH, S, D)                  # [j_full, s, d]

    in_maps = []
    for i in range(NCORES):
        ids_i = np.ascontiguousarray(ids_full[:, i * BPC : (i + 1) * BPC])
        whs = wh_r[:, :, i * DPC : (i + 1) * DPC]          # [NH, S, DPC]
        wh_i = _bf16(whs.transpose(2, 1, 0))               # [d, s, j]
        wd_i = wd[i * VPC : (i + 1) * VPC]                 # [VPC, NH]
        wdT_i = _bf16(wd_i.T.reshape(8, 128, VPC))
        bd_i = _bf16(bd[i * VPC : (i + 1) * VPC].reshape(1, VPC))
        in_maps.append(
            {
                "emb": emb_bf,
                "ids": ids_i,
                "wqT": wqT,
                "wkT": wkT,
                "wvT": wvT,
                "woT": woT,
                "bq": bq_r,
                "bk": bk_r,
                "wh": wh_i,
                "bh": bh_r,
                "wdT": wdT_i,
                "bd": bd_i,
            }
        )
    return in_maps


def _make_runner(nc, in_maps):
    """Build a jitted single-custom-call runner (mirrors bass2jax.run_bass_via_pjrt)
    with device-resident inputs. Returns (run_fn, fetch_outputs_fn)."""
    import jax
    from jax.experimental.shard_map import shard_map
    from jax.sharding import Mesh, PartitionSpec, NamedSharding
    from concourse import bass2jax
    from concourse.bass2jax import _bass_exec_p, partition_id_tensor

    bass2jax.install_neuronx_cc_hook()

    n_cores = len(in_maps)
    partition_name = nc.partition_id_tensor.name if nc.partition_id_tensor else None
    in_names, out_names, out_avals, zero_outs = [], [], [], []
    for alloc in nc.m.functions[0].allocations:
        if not isinstance(alloc, mybir.MemoryLocationSet):
            continue
        name = alloc.memorylocations[0].name
        if alloc.kind == "ExternalInput":
            if name != partition_name:
                in_names.append(name)
        elif alloc.kind == "ExternalOutput":
            out_names.append(name)
            shape = tuple(alloc.tensor_shape)
            dtype = mybir.dt.np(alloc.dtype)
            out_avals.append(jax.core.ShapedArray(shape, dtype))
            zero_outs.append(np.zeros(shape, dtype))
    n_params = len(in_names)
    all_in_names = list(in_names) + list(out_names)
    if partition_name is not None:
        all_in_names.append(partition_name)

    def _body(*args):
        operands = list(args)
        if partition_name is not None:
            operands.append(partition_id_tensor())
        return tuple(
            _bass_exec_p.bind(
                *operands,
                out_avals=tuple(out_avals),
                in_names=tuple(all_in_names),
                out_names=tuple(out_names),
                lowering_input_output_aliases=(),
                sim_require_finite=True,
                sim_require_nnan=True,
                nc=nc,
            )
        )

    devices = jax.devices()[:n_cores]
    mesh = Mesh(np.asarray(devices), ("core",))
    f = jax.jit(
        shard_map(
            _body, mesh=mesh,
            in_specs=(PartitionSpec("core"),) * (n_params + len(out_names)),
            out_specs=(PartitionSpec("core"),) * len(out_names),
            check_rep=False,
        ),
        keep_unused=True,
    )
    concat_in = [
        np.concatenate([np.asarray(in_maps[c][nm]) for c in range(n_cores)], axis=0)
        for nm in in_names
    ]
    concat_zeros = [
        np.zeros((n_cores * z.shape[0], *z.shape[1:]), z.dtype) for z in zero_outs
    ]
    sh = NamedSharding(mesh, PartitionSpec("core"))
    dev_in = [jax.device_put(a, sh) for a in concat_in]
    dev_zero = [jax.device_put(a, sh) for a in concat_zeros]

    def run():
        return jax.block_until_ready(f(*dev_in, *dev_zero))

    def fetch(out_arrs):
        return [
            {
                nm: np.asarray(out_arrs[i]).reshape(n_cores, *out_avals[i].shape)[c]
                for i, nm in enumerate(out_names)
            }
            for c in range(n_cores)
        ]

    return run, fetch


def run_timed(in_maps, iters=11, reps=5):
    """Time the kernel by comparing a repeat=1 program against a repeat=iters
    program (single custom call each; host dispatch cancels in the difference).
    Returns (outputs_per_core, secs_per_exec)."""
    import time

    nc1 = _CACHE.get("nc") or build_program(repeat=1)
    _CACHE["nc"] = nc1
    if "ncN" not in _CACHE or _CACHE.get("ncN_iters") != iters:
        _CACHE["ncN"] = build_program(repeat=iters)
        _CACHE["ncN_iters"] = iters
    run1, fetch1 = _make_runner(nc1, in_maps)
    runN, _ = _make_runner(_CACHE["ncN"], in_maps)

    out1 = run1()  # warm/compile
    runN()         # warm/compile

    best = None
    for _ in range(reps):
        t0 = time.perf_counter()
        run1()
        t1 = time.perf_counter()
        runN()
        t2 = time.perf_counter()
        per = ((t2 - t1) - (t1 - t0)) / (iters - 1)
        best = per if best is None else min(best, per)

    return fetch1(out1), best


def kernel(**inputs) -> np.ndarray:
    if "nc" not in _CACHE:
        _CACHE["nc"] = build_program()
    nc = _CACHE["nc"]
    in_maps = prepare_inputs(inputs)
    res = run_bass_kernel_spmd(nc, in_maps, core_ids=list(range(NCORES)))
    out = np.concatenate([res.results[i]["dec"] for i in range(NCORES)], axis=1)
    return np.ascontiguousarray(out.astype(np.float32))


if __name__ == "__main__":
    # smoke: build only
    build_program()
    print("build ok")

